# revision 18
# baseline (speedup 1.0000x reference)
"""DeepSeekMoE (E=8, top-2, D=2048, H=1408, T=4096) on 8 TRN2 NeuronCores.

Expert-parallel, bf16 datapath. Per core (expert e):
  1. PE warm-up matmuls (flip the HAM clock gate to 8/8 during the initial
     DMA wait), router scores for the core's T/8 token slice via hi/lo bf16
     split (exact top-2 vs the f32 reference), top-2 + sigmoid gate.
     Gate loads: xh on the sync ring, xl on the scalar ring, all chunks
     resident (no buffer-rotation stalls); weights queue FIFO behind them.
  2. AllGather of (i1, i2, g1) as 3 bf16 channels (24KB); ccin + trigger
     ride the gpsimd ring so nothing delays the collective.  While it is
     in flight the core runs the FFN for tokens of its OWN slice routed to
     its own expert (local-first, slots [0, CL)) -- no exchange needed.
  3. After the gather: masks for the other 7 slices (own columns zeroed),
     compact positions via matmul prefix sums, permutation via one-hot
     i16 compares + bf16 matmuls; remote slots [CL, C2) in 512-wide chunks.
  4. FFN per chunk: indirect-gather x rows, PE-transpose, mm1 (fused
     SiLU+b1), mm2 with gate-scale on the vector engine, bf16 rows out.
Host combines: out[idx_e] += y_e + g_e * b2[e] over both slot ranges.
"""

import sys

import numpy as np

sys.path.insert(0, "/opt/trn_rl_repo")

import concourse.bacc as bacc
import concourse.bass as bass
import concourse.mybir as mybir
import concourse.tile as tile
from concourse.bass_utils import run_bass_kernel_spmd
from concourse.masks import make_identity

# Problem shape
T, D, H, E = 4096, 2048, 1408, 8
P = 128
DT, HT, TT = D // P, H // P, T // P      # 16, 11, 32
TS = T // E                              # 512 tokens routed per core
NCH = TS // P                            # 4
CL = 256                                 # local slot capacity (max seen 145)
CR = 1024                                # remote slot capacity (max seen 927)
C2 = CL + CR                             # 1280 total slots
CT2 = C2 // P                            # 10 t-blocks
CW = 512                                 # remote perm/FFN chunk width
NRCH = CR // CW                          # 2 remote chunks
NDG = 4                                  # gate-phase DMA chunks per tensor

F32 = mybir.dt.float32
I32 = mybir.dt.int32
I16 = mybir.dt.int16
U32 = mybir.dt.uint32
BF = mybir.dt.bfloat16
AF = mybir.ActivationFunctionType
OP = mybir.AluOpType


def build_nc():
    nc = bacc.Bacc("TRN2", target_bir_lowering=False)

    # inputs
    xbf = nc.dram_tensor("xbf", [T, D], BF, kind="ExternalInput")
    xh = nc.dram_tensor("xh", [D, TS], BF, kind="ExternalInput")
    xl = nc.dram_tensor("xl", [D, TS], BF, kind="ExternalInput")
    gwh = nc.dram_tensor("gwh", [D, E], BF, kind="ExternalInput")
    gwl = nc.dram_tensor("gwl", [D, E], BF, kind="ExternalInput")
    gbr = nc.dram_tensor("gbr", [P, E], F32, kind="ExternalInput")
    eidv = nc.dram_tensor("eidv", [P, 1], F32, kind="ExternalInput")
    w1t = nc.dram_tensor("w1t", [D, H], BF, kind="ExternalInput")
    w2t = nc.dram_tensor("w2t", [H, D], BF, kind="ExternalInput")
    b1c = nc.dram_tensor("b1c", [P, HT], F32, kind="ExternalInput")
    tri = nc.dram_tensor("tri", [P, P], F32, kind="ExternalInput")
    ut32 = nc.dram_tensor("ut32", [32, 32], F32, kind="ExternalInput")
    onesP = nc.dram_tensor("onesP", [P, 1], F32, kind="ExternalInput")
    ones1 = nc.dram_tensor("ones1", [1, P], F32, kind="ExternalInput")

    # outputs
    yo = nc.dram_tensor("yo", [C2, D], BF, kind="ExternalOutput")
    meta = nc.dram_tensor("meta", [C2, 2], F32, kind="ExternalOutput")
    cnt = nc.dram_tensor("cnt", [1, 2], F32, kind="ExternalOutput")

    with tile.TileContext(nc) as tc:
        with (
            tc.tile_pool(name="wres", bufs=1) as wres,
            tc.tile_pool(name="hTp", bufs=2) as hTp,
            tc.tile_pool(name="xtsp", bufs=6) as xtsp,
            tc.tile_pool(name="xgtp", bufs=2) as xgtp,
            tc.tile_pool(name="xgp", bufs=2) as xgp,
            tc.tile_pool(name="ysp", bufs=2) as ysp,
            tc.tile_pool(name="ohp", bufs=5) as ohp,
            tc.tile_pool(name="small", bufs=1) as small,
            tc.tile_pool(name="rot", bufs=2) as rot,
            tc.tile_pool(name="idxp", bufs=12) as idxp,
            tc.tile_pool(name="pstr", bufs=2, space="PSUM") as pstr,
            tc.tile_pool(name="psh", bufs=2, space="PSUM") as psh,
            tc.tile_pool(name="psy", bufs=4, space="PSUM") as psy,
            tc.tile_pool(name="dram", bufs=1, space="DRAM") as dram,
        ):
            # Router operands first; xh chunks on sync, xl chunks on scalar.
            gwh_sb = small.tile([P, DT, E], BF, name="gwh_sb")
            nc.sync.dma_start(out=gwh_sb[:], in_=gwh.rearrange("(dt p) e -> p dt e", p=P))
            gwl_sb = small.tile([P, DT, E], BF, name="gwl_sb")
            nc.sync.dma_start(out=gwl_sb[:], in_=gwl.rearrange("(dt p) e -> p dt e", p=P))
            gbr_sb = small.tile([P, E], F32, name="gbr_sb")
            nc.sync.dma_start(out=gbr_sb[:], in_=gbr[:, :])

            DG = DT // NDG
            xhv = xh.rearrange("(dt p) t -> p dt t", p=P)
            xlv = xl.rearrange("(dt p) t -> p dt t", p=P)
            xhcs, xlcs = [], []
            for dg in range(NDG):
                c = xtsp.tile([P, DG, TS], BF, tag="xts", name=f"xh_{dg}")
                nc.sync.dma_start(out=c[:], in_=xhv[:, dg * DG:(dg + 1) * DG, :])
                xhcs.append(c)
            for dg in range(NDG):
                c = xtsp.tile([P, DG, TS], BF, tag="xts", name=f"xl_{dg}")
                nc.scalar.dma_start(out=c[:], in_=xlv[:, dg * DG:(dg + 1) * DG, :])
                xlcs.append(c)

            # small consts on the sync ring BEFORE the bulk weight pieces
            eid_sb = small.tile([P, 1], F32, name="eid_sb")
            nc.sync.dma_start(out=eid_sb[:], in_=eidv[:, :])
            b1c_sb = small.tile([P, HT], F32, name="b1c_sb")
            nc.sync.dma_start(out=b1c_sb[:], in_=b1c[:, :])
            tri_sb = small.tile([P, P], F32, name="tri_sb")
            nc.sync.dma_start(out=tri_sb[:], in_=tri[:, :])
            ut32_sb = small.tile([32, 32], F32, name="ut32_sb")
            nc.sync.dma_start(out=ut32_sb[:], in_=ut32[:, :])
            onesP_sb = small.tile([P, 1], F32, name="onesP_sb")
            nc.sync.dma_start(out=onesP_sb[:], in_=onesP[:, :])
            ones1_sb = small.tile([1, P], F32, name="ones1_sb")
            nc.sync.dma_start(out=ones1_sb[:], in_=ones1[:, :])

            # FFN weight tiles; the actual DMAs are emitted after the gate
            # phase (WAW-gated) so they never contend with the gate loads --
            # with all 8 cores pulling, HBM gives each core only ~150GB/s,
            # and 11.5MB of weights at t=0 stretches every core's gate and
            # with it the AllGather barrier.
            w1t_sb = wres.tile([P, DT, H], BF, name="w1t_sb")
            w2t_sb = wres.tile([P, HT, D], BF, name="w2t_sb")
            w1v = w1t.rearrange("(dt p) h -> p dt h", p=P)
            w2v = w2t.rearrange("(ht p) d -> p ht d", p=P)
            W1SPLIT = [0, 384, 768, 1056, 1408]
            iden_sb = small.tile([P, P], F32, name="iden_sb")
            make_identity(nc, iden_sb[:])
            idenb_sb = small.tile([P, P], BF, name="idenb_sb")
            nc.vector.tensor_copy(out=idenb_sb[:], in_=iden_sb[:])

            # ---- PE warm-up: dummy bf16 matmuls on the first xh chunk
            # (arrives ~4us; no dependency on the slower gpsimd iotas).
            with nc.named_scope("warm"):
                warm_ps = psy.tile([P, P], F32, tag="mm", name="warm_ps")
                for _ in range(45):
                    nc.tensor.matmul(out=warm_ps[:], lhsT=xhcs[0][:, 0, 0:P],
                                     rhs=xhcs[0][:, 0, 0:P], start=True, stop=True)

            # ---- phase G: router scores for this core's token slice ----
            # fp32-accurate scores from bf16 planes: x = xh + xl,
            # gw = gh + gl; scores = xh@gh + xh@gl + xl@gh (xl@gl ~2^-18,
            # dropped).  xh passes emitted first so the xl chunks may lag.
            with nc.named_scope("gate"):
                scT_ps = psy.tile([E, TS], F32, tag="mm", name="scT_ps")
                nmm = 0
                for dg in range(NDG):
                    for q in range(DG):
                        dt = dg * DG + q
                        nc.tensor.matmul(
                            out=scT_ps[:], lhsT=gwh_sb[:, dt, :],
                            rhs=xhcs[dg][:, q, :],
                            start=(nmm == 0), stop=False)
                        nmm += 1
                        nc.tensor.matmul(
                            out=scT_ps[:], lhsT=gwl_sb[:, dt, :],
                            rhs=xhcs[dg][:, q, :],
                            start=False, stop=False)
                        nmm += 1
                for dg in range(NDG):
                    for q in range(DG):
                        dt = dg * DG + q
                        nmm += 1
                        nc.tensor.matmul(
                            out=scT_ps[:], lhsT=gwh_sb[:, dt, :],
                            rhs=xlcs[dg][:, q, :],
                            start=False, stop=(nmm == 3 * DT))
                scT = small.tile([E, TS], F32, name="scT")
                nc.vector.tensor_copy(out=scT[:], in_=scT_ps[:])
                sc_all = small.tile([P, NCH, E], F32, name="sc_all")
                scb_ps = psy.tile([P, NCH * E], F32, tag="mm", name="scb_ps")
                for ch in range(NCH):
                    nc.tensor.transpose(out=scb_ps[:, ch * E:(ch + 1) * E],
                                        in_=scT[:, ch * P:(ch + 1) * P],
                                        identity=iden_sb[0:E, 0:E])
                    nc.vector.tensor_add(out=sc_all[:, ch, :],
                                         in0=scb_ps[:, ch * E:(ch + 1) * E],
                                         in1=gbr_sb[:])
                tv = small.tile([P, NCH, 8], F32, name="tv")
                ti = small.tile([P, NCH, 8], U32, name="ti")
                for ch in range(NCH):
                    nc.vector.max_with_indices(tv[:, ch, :], ti[:, ch, :], sc_all[:, ch, :])
                d12 = small.tile([P, NCH], F32, name="d12")
                nc.vector.tensor_sub(out=d12[:], in0=tv[:, :, 0], in1=tv[:, :, 1])
                g1 = small.tile([P, NCH], F32, name="g1")
                nc.scalar.activation(g1[:], d12[:], AF.Sigmoid)
                pack_sb = small.tile([P, NCH, 3], BF, name="pack_sb")
                nc.vector.tensor_copy(out=pack_sb[:, :, 0], in_=ti[:, :, 0])
                nc.vector.tensor_copy(out=pack_sb[:, :, 1], in_=ti[:, :, 1])
                nc.vector.tensor_copy(out=pack_sb[:, :, 2], in_=g1[:])

            # Release the weight transfers now that the gate loads are done
            # (WAW dep via dummy writes into the first piece of each; the
            # later pieces queue FIFO behind them on their rings).
            nc.vector.tensor_copy(out=w1t_sb[0:1, 0, 0:1], in_=d12[0:1, 0:1])
            nc.vector.tensor_copy(out=w2t_sb[0:1, 0, 0:1], in_=d12[0:1, 0:1])
            for q in range(4):
                nc.scalar.dma_start(out=w1t_sb[:, :, W1SPLIT[q]:W1SPLIT[q + 1]],
                                    in_=w1v[:, :, W1SPLIT[q]:W1SPLIT[q + 1]])
            for q in range(4):
                nc.sync.dma_start(out=w2t_sb[:, :, q * 512:(q + 1) * 512],
                                  in_=w2v[:, :, q * 512:(q + 1) * 512])

            # shared helpers (gpsimd iotas early, before the ccin wait)
            iota32 = small.tile([P, TT], F32, name="iota32")
            nc.gpsimd.iota(iota32[:], pattern=[[1, TT]], base=0, channel_multiplier=0,
                           allow_small_or_imprecise_dtypes=True)
            iota_p = small.tile([P, 1], F32, name="iota_p")
            nc.gpsimd.iota(iota_p[:], pattern=[[1, 1]], base=0, channel_multiplier=1,
                           allow_small_or_imprecise_dtypes=True)
            crow16 = small.tile([P, C2], I16, name="crow16")
            nc.gpsimd.iota(crow16[:], pattern=[[1, C2]], base=0, channel_multiplier=0)
            eid4 = small.tile([P, 1], F32, name="eid4")
            nc.vector.tensor_scalar(eid4[:], eid_sb[:], 4.0, scalar2=None, op0=OP.mult)

            # ---- all-gather routing info across the 8 cores (bf16) ----
            # ccin + trigger on the gpsimd ring: nothing else is queued
            # there this early, so the collective fires as soon as the
            # pack is written.  rtab + remote gathers follow on the same
            # ring later (they need the CC result anyway).
            with nc.named_scope("cc"):
                ccin = dram.tile([TS, 3], BF, name="ccin")
                ccout = dram.tile([T, 3], BF, addr_space="Shared", name="ccout")
                nc.gpsimd.dma_start(
                    out=ccin.rearrange("(c p) w -> p c w", p=P),
                    in_=pack_sb[:],
                )
                nc.gpsimd.collective_compute(
                    "AllGather",
                    OP.bypass,
                    replica_groups=[list(range(E))],
                    ins=[ccin[:, :]],
                    outs=[ccout[:, :]],
                )

            meta_sb = small.tile([P, CT2, 2], F32, name="meta_sb")
            cnt_sb = small.tile([1, 2], F32, name="cnt_sb")
            idx_ts = [None] * CT2

            # ---- generic building blocks ----
            def emit_compact(mask, width, utN, posf, base, sent, tag):
                """posf[t] = base + rank(t) if mask else sent (matmul prefix sums)."""
                csT_ps = psy.tile([width, 1], F32, tag="mm", name=f"csT_{tag}")
                nc.tensor.matmul(out=csT_ps[:], lhsT=mask[:], rhs=onesP_sb[:], start=True, stop=True)
                csT = small.tile([width, 1], F32, name=f"csTs_{tag}")
                nc.vector.tensor_copy(out=csT[:], in_=csT_ps[:])

                carry_ps = psy.tile([1, width], F32, tag="mm", name=f"carry_{tag}")
                nc.tensor.matmul(out=carry_ps[:], lhsT=csT[:], rhs=utN, start=True, stop=True)
                carry = small.tile([1, width], F32, name=f"carrys_{tag}")
                nc.vector.tensor_copy(out=carry[:], in_=carry_ps[:])

                cnt_ps = psy.tile([1, 1], F32, tag="mm", name=f"cnt_{tag}")
                nc.tensor.matmul(out=cnt_ps[:], lhsT=csT[:], rhs=onesP_sb[0:width, :], start=True, stop=True)
                nc.vector.tensor_copy(out=cnt_sb[0:1, tag:tag + 1], in_=cnt_ps[:])

                pos_ps = psy.tile([P, width], F32, tag="mm", name=f"pos_{tag}")
                nc.tensor.matmul(out=pos_ps[:], lhsT=tri_sb[:], rhs=mask[:], start=True, stop=False)
                nc.tensor.matmul(out=pos_ps[:], lhsT=ones1_sb[:], rhs=carry[:], start=False, stop=True)

                nc.vector.tensor_mul(out=posf[:], in0=pos_ps[:], in1=mask[:])
                pc = small.tile([P, width], F32, name=f"pc_{tag}")
                nc.vector.tensor_scalar(pc[:], mask[:], float(base - sent), scalar2=float(sent),
                                        op0=OP.mult, op1=OP.add)
                nc.vector.tensor_add(out=posf[:], in0=posf[:], in1=pc[:])

            def emit_perm(base, width, tgv, posf, jn, tag):
                """Build idx/meta for slots [base, base+width) from token tiles 0..jn."""
                cpT_ps = psy.tile([3, width], F32, tag="mm", name=f"cpT_{tag}")
                for j in range(jn):
                    oh = ohp.tile([P, width], BF, tag="oh", name=f"oh_{tag}_{j}")
                    nc.vector.tensor_scalar(
                        oh[:], crow16[:, base:base + width], posf[:, j:j + 1],
                        scalar2=None, op0=OP.is_equal)
                    nc.tensor.matmul(out=cpT_ps[:], lhsT=tgv[:, j, :], rhs=oh[:],
                                     start=(j == 0), stop=(j == jn - 1))
                cpT_sb = rot.tile([3, width], F32, tag="cpT", name=f"cpTs_{tag}")
                nc.vector.tensor_copy(out=cpT_sb[:], in_=cpT_ps[:])
                for i in range(width // P):
                    t = base // P + i
                    tr3_ps = psy.tile([P, 3], F32, tag="mm", name=f"tr3_{t}")
                    nc.tensor.transpose(out=tr3_ps[:], in_=cpT_sb[:, i * P:(i + 1) * P],
                                        identity=iden_sb[0:3, 0:3])
                    dec = idxp.tile([P, 1], F32, tag="dec", name=f"dec_{t}")
                    nc.vector.tensor_scalar(dec[:], tr3_ps[:, 0:1], 128.0,
                                            scalar2=None, op0=OP.mult)
                    nc.vector.tensor_add(out=dec[:], in0=dec[:], in1=tr3_ps[:, 1:2])
                    nc.vector.tensor_copy(out=meta_sb[:, t, 0:1], in_=dec[:])
                    nc.vector.tensor_copy(out=meta_sb[:, t, 1:2], in_=tr3_ps[:, 2:3])
                    idx_t = idxp.tile([P, 1], I32, tag="idx", name=f"idx_{t}")
                    nc.vector.tensor_copy(out=idx_t[:], in_=dec[:])
                    idx_ts[t] = idx_t

            def emit_gather_txp(t, xgT_k, i):
                xg = xgp.tile([P, D], BF, tag="xg", name=f"xg_{t}")
                nc.gpsimd.indirect_dma_start(
                    out=xg[:], out_offset=None,
                    in_=xbf[:, :],
                    in_offset=bass.IndirectOffsetOnAxis(ap=idx_ts[t][:, 0:1], axis=0),
                )
                for g in range(DT // 4):
                    tr = pstr.tile([P, 4 * P], BF, tag="tr", name=f"tr_{t}_{g}")
                    for q in range(4):
                        dt = g * 4 + q
                        nc.tensor.transpose(out=tr[:, q * P:(q + 1) * P],
                                            in_=xg[:, dt * P:(dt + 1) * P],
                                            identity=idenb_sb[:])
                    nc.scalar.activation(
                        xgT_k[:, g * 4:(g + 1) * 4, i * P:(i + 1) * P],
                        tr.rearrange("p (q c) -> p q c", q=4),
                        AF.Copy)

            def emit_mm1(width, xgT_k, hT_k):
                for ht in range(HT):
                    hp = psh.tile([P, width], F32, tag="mm", name=f"hp_{id(xgT_k)}_{ht}")
                    for dt in range(DT):
                        nc.tensor.matmul(
                            out=hp[:],
                            lhsT=w1t_sb[:, dt, ht * P:(ht + 1) * P],
                            rhs=xgT_k[:, dt, :],
                            start=(dt == 0), stop=(dt == DT - 1),
                        )
                    nc.scalar.activation(hT_k[:, ht, :], hp[:],
                                         AF.Silu, bias=b1c_sb[:, ht:ht + 1])

            def emit_mm2(t, hT_k, i):
                ysb = ysp.tile([P, D], BF, tag="ysb", name=f"ysb_{t}")
                for dch in range(4):
                    yp = psy.tile([P, 512], F32, tag="mm", name=f"yp_{t}_{dch}")
                    for ht in range(HT):
                        nc.tensor.matmul(
                            out=yp[:],
                            lhsT=hT_k[:, ht, i * P:(i + 1) * P],
                            rhs=w2t_sb[:, ht, dch * 512:(dch + 1) * 512],
                            start=(ht == 0), stop=(ht == HT - 1),
                        )
                    nc.vector.tensor_scalar(ysb[:, dch * 512:(dch + 1) * 512], yp[:],
                                            meta_sb[:, t, 1:2], scalar2=None, op0=OP.mult)
                nc.sync.dma_start(out=yo[t * P:(t + 1) * P, :], in_=ysb[:])

            # ---- LOCAL pass: own-slice tokens routed to our own expert ----
            # Entirely independent of the AllGather -- fills the CC window.
            with nc.named_scope("lroute"):
                ti0f = small.tile([P, NCH], F32, name="ti0f")
                nc.vector.tensor_copy(out=ti0f[:], in_=ti[:, :, 0])
                ti1f = small.tile([P, NCH], F32, name="ti1f")
                nc.vector.tensor_copy(out=ti1f[:], in_=ti[:, :, 1])
                ebL = eid_sb[:, 0:1].to_broadcast([P, NCH])
                lm1 = small.tile([P, NCH], F32, name="lm1")
                lm2 = small.tile([P, NCH], F32, name="lm2")
                lmask = small.tile([P, NCH], F32, name="lmask")
                lgate = small.tile([P, NCH], F32, name="lgate")
                ldm = small.tile([P, NCH], F32, name="ldm")
                nc.vector.tensor_tensor(out=lm1[:], in0=ti0f[:], in1=ebL, op=OP.is_equal)
                nc.vector.tensor_tensor(out=lm2[:], in0=ti1f[:], in1=ebL, op=OP.is_equal)
                nc.vector.tensor_add(out=lmask[:], in0=lm1[:], in1=lm2[:])
                nc.vector.tensor_sub(out=ldm[:], in0=lm1[:], in1=lm2[:])
                nc.vector.tensor_mul(out=ldm[:], in0=ldm[:], in1=g1[:])
                nc.vector.tensor_add(out=lgate[:], in0=ldm[:], in1=lm2[:])
                posf_l = small.tile([P, NCH], F32, name="posf_l")
                emit_compact(lmask, NCH, ut32_sb[0:NCH, 0:NCH], posf_l, 0, C2, 0)
                tgv_l = small.tile([P, NCH, 3], BF, name="tgv_l")
                hi_l = small.tile([P, NCH], F32, name="hi_l")
                nc.vector.tensor_tensor(out=hi_l[:], in0=iota32[:, 0:NCH],
                                        in1=eid4[:, 0:1].to_broadcast([P, NCH]), op=OP.add)
                nc.vector.tensor_copy(out=tgv_l[:, :, 0], in_=hi_l[:])
                nc.vector.tensor_copy(out=tgv_l[:, :, 1], in_=iota_p[:].to_broadcast([P, NCH]))
                nc.vector.tensor_copy(out=tgv_l[:, :, 2], in_=lgate[:])
            with nc.named_scope("lperm"):
                emit_perm(0, CL, tgv_l, posf_l, NCH, "L")
            # full-width (CW) tiles so every slot in the tag ring has the
            # same size; the local pass only uses the first CL columns.
            xgT_lf = xgtp.tile([P, DT, CW], BF, tag="xgT", name="xgT_l")
            xgT_l = xgT_lf[:, :, 0:CL]
            with nc.named_scope("lgtx"):
                for i in range(CL // P):
                    emit_gather_txp(i, xgT_l, i)
            hT_lf = hTp.tile([P, HT, CW], BF, tag="hT", name="hT_l")
            hT_l = hT_lf[:, :, 0:CL]
            with nc.named_scope("lmm1"):
                emit_mm1(CL, xgT_l, hT_l)
            with nc.named_scope("lmm2"):
                for i in range(CL // P):
                    emit_mm2(i, hT_l, i)

            # ---- REMOTE pass: tokens from the other 7 slices ----
            with nc.named_scope("route"):
                rtab = small.tile([P, TT, 3], BF, name="rtab")
                nc.gpsimd.dma_start(out=rtab[:], in_=ccout.rearrange("(tt p) w -> p tt w", p=P))
                rt32 = small.tile([P, TT, 3], F32, name="rt32")
                nc.vector.tensor_copy(out=rt32[:], in_=rtab[:])
                m1 = small.tile([P, TT], F32, name="m1")
                m2 = small.tile([P, TT], F32, name="m2")
                mask_all = small.tile([P, TT], F32, name="mask_all")
                gate_all = small.tile([P, TT], F32, name="gate_all")
                dmm = small.tile([P, TT], F32, name="dmm")
                eb = eid_sb[:, 0:1].to_broadcast([P, TT])
                nc.vector.tensor_tensor(out=m1[:], in0=rt32[:, :, 0], in1=eb, op=OP.is_equal)
                nc.vector.tensor_tensor(out=m2[:], in0=rt32[:, :, 1], in1=eb, op=OP.is_equal)
                nc.vector.tensor_add(out=mask_all[:], in0=m1[:], in1=m2[:])
                nc.vector.tensor_sub(out=dmm[:], in0=m1[:], in1=m2[:])
                nc.vector.tensor_mul(out=dmm[:], in0=dmm[:], in1=rt32[:, :, 2])
                nc.vector.tensor_add(out=gate_all[:], in0=dmm[:], in1=m2[:])
                # zero our own slice's columns (handled by the local pass)
                own = small.tile([P, TT], F32, name="own")
                ow2 = small.tile([P, TT], F32, name="ow2")
                nc.vector.tensor_tensor(out=own[:], in0=iota32[:],
                                        in1=eid4[:, 0:1].to_broadcast([P, TT]), op=OP.is_ge)
                nc.vector.tensor_scalar(ow2[:], iota32[:], -4.0, scalar2=None, op0=OP.add)
                nc.vector.tensor_tensor(out=ow2[:], in0=ow2[:],
                                        in1=eid4[:, 0:1].to_broadcast([P, TT]), op=OP.is_lt)
                nc.vector.tensor_mul(out=own[:], in0=own[:], in1=ow2[:])
                nc.vector.tensor_scalar(own[:], own[:], -1.0, scalar2=1.0,
                                        op0=OP.mult, op1=OP.add)
                nc.vector.tensor_mul(out=mask_all[:], in0=mask_all[:], in1=own[:])
                posf = small.tile([P, TT], F32, name="posf")
            with nc.named_scope("compact"):
                emit_compact(mask_all, TT, ut32_sb[:, :], posf, CL, C2, 1)
                tgv = small.tile([P, TT, 3], BF, name="tgv")
                nc.vector.tensor_copy(out=tgv[:, :, 0], in_=iota32[:])
                nc.vector.tensor_copy(out=tgv[:, :, 1], in_=iota_p[:].to_broadcast([P, TT]))
                nc.vector.tensor_copy(out=tgv[:, :, 2], in_=gate_all[:])

            for k in range(NRCH):
                with nc.named_scope(f"perm{k}"):
                    emit_perm(CL + k * CW, CW, tgv, posf, TT, f"R{k}")

            TPC = CW // P  # t-blocks per chunk
            hT_r = [None] * NRCH
            for k in range(NRCH):
                if k >= 1:
                    with nc.named_scope(f"mm2_{k - 1}"):
                        for i in range(TPC):
                            emit_mm2(CL // P + (k - 1) * TPC + i, hT_r[k - 1], i)
                xgT_k = xgtp.tile([P, DT, CW], BF, tag="xgT", name=f"xgT_{k}")
                with nc.named_scope(f"gtx{k}"):
                    for i in range(TPC):
                        emit_gather_txp(CL // P + k * TPC + i, xgT_k, i)
                hT_r[k] = hTp.tile([P, HT, CW], BF, tag="hT", name=f"hT_{k}")
                with nc.named_scope(f"mm1_{k}"):
                    emit_mm1(CW, xgT_k, hT_r[k])
            with nc.named_scope(f"mm2_{NRCH - 1}"):
                for i in range(TPC):
                    emit_mm2(CL // P + (NRCH - 1) * TPC + i, hT_r[NRCH - 1], i)

            # cnt/meta on the scalar ring (idle after w1) so the final yo
            # writes on the sync ring drain without anything behind them.
            nc.scalar.dma_start(out=cnt[0:1, 0:2], in_=cnt_sb[:])
            nc.scalar.dma_start(out=meta.rearrange("(ct p) w -> p ct w", p=P),
                                in_=meta_sb[:])

    nc.compile()
    return nc


_NC_CACHE = {}


def _get_nc():
    if "nc" not in _NC_CACHE:
        _NC_CACHE["nc"] = build_nc()
    return _NC_CACHE["nc"]


def _prep_inputs(x, gate_w, gate_b, bias, w1, b1, w2, b2):
    import ml_dtypes
    bf16 = ml_dtypes.bfloat16
    xf = np.ascontiguousarray(x.reshape(T, D).astype(np.float32))
    xbf = np.ascontiguousarray(xf.astype(bf16))
    gwtf = gate_w.astype(np.float32).T          # [D, E]
    gwh = gwtf.astype(bf16)
    gwl = (gwtf - gwh.astype(np.float32)).astype(bf16)
    gbr = np.ascontiguousarray(
        np.broadcast_to((gate_b + bias).astype(np.float32), (P, E)))
    tri = np.triu(np.ones((P, P), dtype=np.float32), 1)
    ut = np.triu(np.ones((32, 32), dtype=np.float32), 1)
    onesP = np.ones((P, 1), dtype=np.float32)
    ones1 = np.ones((1, P), dtype=np.float32)
    in_maps = []
    for e in range(E):
        xsl = xf[e * TS:(e + 1) * TS].T          # [D, TS] f32
        xsh = xsl.astype(bf16)
        xslo = (xsl - xsh.astype(np.float32)).astype(bf16)
        in_maps.append({
            "xbf": xbf,
            "xh": np.ascontiguousarray(xsh),
            "xl": np.ascontiguousarray(xslo),
            "gwh": np.ascontiguousarray(gwh),
            "gwl": np.ascontiguousarray(gwl),
            "gbr": gbr,
            "eidv": np.full((P, 1), float(e), dtype=np.float32),
            "w1t": np.ascontiguousarray(w1[e].astype(np.float32).T.astype(bf16)),
            "w2t": np.ascontiguousarray(w2[e].astype(np.float32).T.astype(bf16)),
            "b1c": np.ascontiguousarray(b1[e].astype(np.float32).reshape(HT, P).T),
            "tri": tri,
            "ut32": ut,
            "onesP": onesP,
            "ones1": ones1,
        })
    return in_maps


def _run(inputs, trace=False):
    x = np.asarray(inputs["x"], dtype=np.float32)
    gate_w = np.asarray(inputs["gate_w"], dtype=np.float32)
    gate_b = np.asarray(inputs["gate_b"], dtype=np.float32)
    bias = np.asarray(inputs["bias"], dtype=np.float32)
    w1 = np.asarray(inputs["w1"], dtype=np.float32)
    b1 = np.asarray(inputs["b1"], dtype=np.float32)
    w2 = np.asarray(inputs["w2"], dtype=np.float32)
    b2 = np.asarray(inputs["b2"], dtype=np.float32)

    in_maps = _prep_inputs(x, gate_w, gate_b, bias, w1, b1, w2, b2)
    nc = _get_nc()
    kwargs = {}
    if trace:
        kwargs = {"trace": True, "trace_cores": list(range(E))}
    res = run_bass_kernel_spmd(nc, in_maps, core_ids=list(range(E)), **kwargs)

    out = np.zeros((T, D), dtype=np.float32)
    for e in range(E):
        r = res.results[e]
        nl = int(round(float(r["cnt"][0, 0])))
        nr = int(round(float(r["cnt"][0, 1])))
        assert 0 <= nl <= CL, f"expert {e} local count {nl} exceeds {CL}"
        assert 0 <= nr <= CR, f"expert {e} remote count {nr} exceeds {CR}"
        for lo, n in ((0, nl), (CL, nr)):
            if n == 0:
                continue
            idx = r["meta"][lo:lo + n, 0].astype(np.int64)
            g = r["meta"][lo:lo + n, 1].astype(np.float32)
            out[idx] += r["yo"][lo:lo + n].astype(np.float32) + g[:, None] * b2[e][None, :]
    return out.reshape(x.shape), res


def kernel(**inputs) -> np.ndarray:
    out, _ = _run(inputs, trace=False)
    return out
